# revision 2
# baseline (speedup 1.0000x reference)
"""Block-diagonal linear kernel for 8 TRN2 NeuronCores.

Problem: x [4096, 8192] fp32, blocks [64, 128, 128] fp32,
out[b, n*128+r] = sum_c x[b, n*128+c] * blocks[n, r, c].

Sharding: block-parallel (expert-style). Core k owns blocks 8k..8k+7, the
matching x column-slice x[:, 1024k:1024(k+1)] and output column-slice
out[:, 1024k:1024(k+1)]. Communication-free.

The kernel is HBM-traffic bound (~330-345 GB/s sustained mixed R/W per
core), so the streams are quantized to 1 byte/element:

  x stream (int8): host quantizes xT per input column c with
    s_c = max_b|x| / 127 and folds s_c into the weights
    (w'[c,i,r] = blocks[n,r,c] * s_c), so the device only casts
    int8 -> fp16 (exact) and runs a plain fp16 matmul; psum holds the
    true output. Per-column scaling keeps the quantization error at
    ~0.9% L2 (gate is 2e-2).

  out stream (int8): host predicts the output std per (block, row) from
    the folded weight norms and the quantized-x second moments
    (var_o[i,r] = sum_c w'^2 * E[q^2]), sets s_o = ALPHA*std/127, and the
    device folds 1/s_o into the psum->SBUF pass (ACT/DVE per-partition
    scale + saturating RNE int8 cast). Host multiplies s_o back. Adds
    ~1.0% quantization noise; measured total ~1.3-1.4% L2.

Device per block i: load xq slab [128, 4096] int8 (512 KiB, SP ring),
DVE tensor_copy int8->fp16 (2x mode), 8x matmul(psum[128, 512] fp16),
psum pairs [128, 1024] scaled+cast to int8 out slab (ACT + DVE split),
store [128, 4096] int8 (ACT ring).
"""

import numpy as np

import concourse.mybir as mybir
import concourse.tile as tile
from concourse import bacc, bass_utils

N_CORES = 8
N_BLOCKS = 64
BLK = 128                      # block rows/cols
BATCH = 4096
D = N_BLOCKS * BLK             # 8192
BPC = N_BLOCKS // N_CORES      # 8 blocks per core
CLS = BPC * BLK                # 1024: column-slice width per core
NCHUNK = 512                   # matmul moving-dim (fp32 PSUM bank limit)
NB = BATCH // NCHUNK           # 8 batch chunks
PAIR = 2 * NCHUNK              # psum copy granularity (2 banks)
NP = BATCH // PAIR             # 4 psum pairs per slab

OUT_INT8 = True                # False: fp16 out stream (safer, ~12.5 MiB/core)
ALPHA = 4.25                   # out-scale headroom in sigmas (int8 clips above)

_CACHE = {}


def _emit_body(nc, pools, w_sb, ro_sb, xt, outt):
    """One full pass over the core's shard."""
    f32 = mybir.dt.float32
    fp16 = mybir.dt.float16
    odt = mybir.dt.int8 if OUT_INT8 else fp16
    xqpool, xfpool, opool, pspool = pools
    for i in range(BPC):
        xq_sb = xqpool.tile([BLK, BATCH], mybir.dt.int8)
        nc.sync.dma_start(out=xq_sb, in_=xt[i * BLK : (i + 1) * BLK, :])
        xf_sb = xfpool.tile([BLK, BATCH], fp16)
        # int8 -> fp16 decompress (exact); single-src SBUF op -> DVE 2x mode
        nc.vector.tensor_copy(out=xf_sb, in_=xq_sb)
        o_sb = opool.tile([BLK, BATCH], odt)
        for p in range(NP):
            ps = pspool.tile([BLK, PAIR], f32)
            for h in range(2):
                j = 2 * p + h
                nc.tensor.matmul(
                    ps[:, h * NCHUNK : (h + 1) * NCHUNK],
                    lhsT=w_sb[:, i, :],
                    rhs=xf_sb[:, j * NCHUNK : (j + 1) * NCHUNK],
                    start=True,
                    stop=True,
                )
            osl = o_sb[:, p * PAIR : (p + 1) * PAIR]
            if OUT_INT8:
                # out = round_sat_i8(psum * (1/s_o)); per-partition scale.
                # 3 of 4 pairs on ACT (closer to PSUM), 1 on DVE to balance
                # against its decompress load.
                if p < 3:
                    nc.scalar.activation(
                        osl, ps, mybir.ActivationFunctionType.Copy,
                        scale=ro_sb[:, i : i + 1],
                    )
                else:
                    nc.vector.tensor_scalar_mul(osl, ps, ro_sb[:, i : i + 1])
            else:
                nc.scalar.copy(osl, ps)
        nc.scalar.dma_start(out=outt[i * BLK : (i + 1) * BLK, :], in_=o_sb)


def _build_bass(iters: int = 1, loop_iters: int = 0, loop_unroll: int = 4):
    """One SPMD program; every core runs it on its own shard.

    iters > 1 (python-unrolled) or loop_iters > 0 (device For_i around
    loop_unroll python-unrolled passes) repeat the body with identical I/O —
    used only for timing via the slope method (axon dispatch overhead,
    ~80 ms, dominates any single wall-clock call).
    """
    nc = bacc.Bacc("TRN2", debug=False, num_devices=N_CORES, target_bir_lowering=False)
    fp16 = mybir.dt.float16
    odt = mybir.dt.int8 if OUT_INT8 else fp16
    xt = nc.dram_tensor("xt", [CLS, BATCH], mybir.dt.int8, kind="ExternalInput").ap()
    # weights arrive host-swizzled as [c, i, r], scale-folded, fp16
    wt = nc.dram_tensor("wt", [BLK, BPC, BLK], fp16, kind="ExternalInput").ap()
    # reciprocal out scales [r, i] fp32 (unused buffer when OUT_INT8=False)
    rt = nc.dram_tensor("rt", [BLK, BPC], mybir.dt.float32, kind="ExternalInput").ap()
    outt = nc.dram_tensor("outt", [CLS, BATCH], odt, kind="ExternalOutput").ap()

    with tile.TileContext(nc) as tc:
        with (
            tc.tile_pool(name="w", bufs=1) as wpool,
            tc.tile_pool(name="xq", bufs=3) as xqpool,
            tc.tile_pool(name="xf", bufs=2) as xfpool,
            tc.tile_pool(name="xout", bufs=3) as opool,
            tc.tile_pool(name="ps", bufs=4, space="PSUM") as pspool,
        ):
            w_sb = wpool.tile([BLK, BPC, BLK], fp16)
            nc.scalar.dma_start(out=w_sb, in_=wt)
            ro_sb = wpool.tile([BLK, BPC], mybir.dt.float32)
            nc.scalar.dma_start(out=ro_sb, in_=rt)

            pools = (xqpool, xfpool, opool, pspool)
            if loop_iters > 0:
                with tc.For_i(0, loop_iters, 1):
                    for _ in range(loop_unroll):
                        _emit_body(nc, pools, w_sb, ro_sb, xt, outt)
            else:
                for _ in range(iters):
                    _emit_body(nc, pools, w_sb, ro_sb, xt, outt)
    nc.compile()
    return nc


def _get_bass():
    if "nc" not in _CACHE:
        _CACHE["nc"] = _build_bass()
    return _CACHE["nc"]


def _quantize_host(x: np.ndarray, blocks: np.ndarray):
    """Per-column int8 x; scales folded into fp16 weights; out-scale predict."""
    xT = np.ascontiguousarray(x.T)                       # [D, BATCH] fp32
    s_c = np.abs(xT).max(axis=1) / 127.0                 # [D]
    np.maximum(s_c, 1e-30, out=s_c)
    q_x = np.rint(xT / s_c[:, None]).astype(np.int8)     # RNE, no clip needed
    # folded weights per core: w'[c, i, r] = blocks[8k+i, r, c] * s_c[f]
    scl = s_c.reshape(N_BLOCKS, BLK)                     # [n, c]
    w_folded = blocks.transpose(0, 2, 1) * scl[:, :, None]   # [n, c, r] fp32
    # predicted out std per (n, r): sum_c w'^2 * E[q^2]
    eq2 = (q_x.astype(np.float32) ** 2).mean(axis=1).reshape(N_BLOCKS, BLK)
    var_o = np.einsum("ncr,nc->nr", w_folded.astype(np.float32) ** 2, eq2)
    s_o = ALPHA * np.sqrt(var_o) / 127.0                 # [n, r]
    np.maximum(s_o, 1e-30, out=s_o)
    return q_x, w_folded.astype(np.float16), s_o


def _make_in_maps(x: np.ndarray, blocks: np.ndarray):
    x = np.asarray(x, np.float32)
    blocks = np.asarray(blocks, np.float32)
    q_x, w_folded, s_o = _quantize_host(x, blocks)
    ro = (1.0 / s_o).astype(np.float32)                  # [n, r]
    in_maps = []
    for k in range(N_CORES):
        wt = np.ascontiguousarray(
            w_folded[BPC * k : BPC * (k + 1)].transpose(1, 0, 2)  # [c, i, r]
        )
        rt = np.ascontiguousarray(ro[BPC * k : BPC * (k + 1)].T)  # [r, i]
        in_maps.append(
            {"xt": q_x[CLS * k : CLS * (k + 1)], "wt": wt, "rt": rt}
        )
    return in_maps, s_o


def _gather(results, s_o):
    out = np.empty((BATCH, D), dtype=np.float32)
    for k in range(N_CORES):
        o = results[k]["outt"].T.astype(np.float32, copy=False)  # [BATCH, CLS]
        if OUT_INT8:
            o = o * s_o[BPC * k : BPC * (k + 1)].reshape(CLS)[None, :]
        out[:, CLS * k : CLS * (k + 1)] = o
    return out


def kernel(x: np.ndarray, blocks: np.ndarray) -> np.ndarray:
    nc = _get_bass()
    in_maps, s_o = _make_in_maps(x, blocks)
    try:
        res = bass_utils.run_bass_kernel_spmd(
            nc, in_maps, core_ids=list(range(N_CORES))
        )
    except Exception:
        # The axon relay occasionally throws a transient
        # NRT_EXEC_UNIT_UNRECOVERABLE on a fresh process; the backend
        # usually recovers. Best-effort reset + one retry.
        try:
            import jax

            jax.clear_backends()
        except Exception:
            pass
        res = bass_utils.run_bass_kernel_spmd(
            nc, in_maps, core_ids=list(range(N_CORES))
        )
    return _gather(res.results, s_o)
